# revision 2
# baseline (speedup 1.0000x reference)
"""BioZorro dense-transformer kernel for 8 Trainium2 NeuronCores — v2.

Strategy: data-parallel over batch (4 pairs of cores) x sequence-parallel
within each pair (520 tokens per core).  Per layer each core LayerNorms its
local tokens, AllGathers the normalized activations across the pair (bf16,
~1MB per core — 4x smaller than gathering k^T+v in f32), then recomputes
q/k/v locally: q for its local 520 queries, k/v for ALL 1040 keys (the PE
has idle capacity; this trades cheap local matmuls for collective bytes).
Attention, Wo and the GEGLU FFN are local.  LN gammas are folded into the
following weight matrices on the host; the final LN gamma is applied in the
host epilogue.  Weights are pre-tiled on the host into contiguous bf16
blocks streamed via HWDGE on the SP queue; the collective and its unpack
DMAs own the Pool queue so they never block compute queues.

Matmul inputs are bf16 (fp32 PSUM accumulate); the residual stream is fp32.
The pooled-attention + contrastive-loss epilogue runs on host (its mask
reduces to a uniform mean over v).
"""

import numpy as np

B = 4
N_SPL = 1024
NF = 16
T = N_SPL + NF            # 1040 tokens per batch element
T_LOC = T // 2            # 520 per core
DIM = 1024
NH = 16
DH = 64
DEPTH = 6
FF = 2730
FF_PAD = 2816             # 22 * 128
FFC = FF_PAD // 128       # 22
VCOLS = NH * (DH + 1)     # 1040: per head 64 v cols + 1 ones col
VOCAB = 20000
EMB_EXT = VOCAB + NF

# free-dim (token-stream) chunks: a psum bank holds 512 f32
TC = [(0, 512), (512, 8)]
# key-partition chunks within half0 (for v production)
TCH = [(0, 128), (128, 128), (256, 128), (384, 128), (512, 8)]
TCH1 = [(0, 128), (128, 128), (256, 128), (384, 120)]   # half1 minus fusion
# main-pass key chunks in GLOBAL order: (half, offset, size); excludes the
# 16 fusion keys (half1 positions 504..520) handled by the fusion pre-pass
JCH = [(0, 0, 128), (0, 128, 128), (0, 256, 128), (0, 384, 128), (0, 512, 8),
       (1, 0, 128), (1, 128, 128), (1, 256, 128), (1, 384, 120)]
# v-column chunks
VC = [(0, 512), (512, 512), (1024, 16)]

XGB = 128 * T_LOC         # elements per (dim-chunk) block of gathered xn

_CACHE = {}


def _split_multi_waits(nc):
    """This container's walrus supports only one sync wait per instruction;
    split any instruction carrying more by inserting same-engine NoOps."""
    import bass_rust
    import concourse.mybir as mybir

    for bb in nc.main_func.blocks:
        insts = list(bb.instructions)
        out = []
        changed = False
        for inst in insts:
            si = getattr(inst, "sync_info", None)
            waits = list(si.on_wait) if si is not None and si.on_wait else []
            if len(waits) > 1:
                for k, w in enumerate(waits[:-1]):
                    nop = bass_rust.InstNoOp(
                        name=f"{inst.name}-wsplit{k}",
                        engine=inst.engine,
                        sync_info=mybir.SyncInfo(on_wait=[w], on_update=[]),
                    )
                    nc.register_instruction(nop)
                    out.append(nop)
                inst.sync_info = mybir.SyncInfo(
                    on_wait=[waits[-1]], on_update=list(si.on_update or [])
                )
                changed = True
            out.append(inst)
        if changed:
            bb.instructions = out


def build_program(depth=DEPTH):
    import concourse.bass as bass
    import concourse.mybir as mybir
    import concourse.tile as tile
    from contextlib import ExitStack

    F32 = mybir.dt.float32
    F32R = mybir.dt.float32r
    BF16 = mybir.dt.bfloat16
    AF = mybir.ActivationFunctionType
    OP = mybir.AluOpType
    AX = mybir.AxisListType

    nc = bass.Bass()

    # dim-major pre-scaled token embeddings: [p, dc, tok]
    tokr = nc.declare_dram_parameter("tokr", [128, 8, T_LOC], F32R, isOutput=False)
    onesin = nc.declare_dram_parameter("onesin", [128, 128], F32R, isOutput=False)
    cmeanin = nc.declare_dram_parameter("cmeanin", [128, 2], F32R, isOutput=False)
    fmaskin = nc.declare_dram_parameter("fmaskin", [128, 1], F32R, isOutput=False)
    # pre-tiled weights; host layout per (l, chunk): [128, cols] contiguous
    wq = nc.declare_dram_parameter("wq", [depth, 8, 128, DIM], BF16, isOutput=False)
    wk = nc.declare_dram_parameter("wk", [depth, 8, 128, DIM], BF16, isOutput=False)
    wv = nc.declare_dram_parameter("wv", [depth, 8, 128, VCOLS], BF16, isOutput=False)
    wo = nc.declare_dram_parameter("wo", [depth, 8, 128, DIM], BF16, isOutput=False)
    w1x = nc.declare_dram_parameter("w1x", [depth, FFC, 128, DIM], BF16, isOutput=False)
    w1g = nc.declare_dram_parameter("w1g", [depth, FFC, 128, DIM], BF16, isOutput=False)
    w2 = nc.declare_dram_parameter("w2", [depth, FFC, 128, DIM], BF16, isOutput=False)
    osum = nc.declare_dram_parameter("osum", [DIM], F32, isOutput=True)

    rg = [[0, 1], [2, 3], [4, 5], [6, 7]]

    with tile.TileContext(nc) as tc, nc.allow_low_precision(reason="bf16 compute"), \
         ExitStack() as es:
        outer = es.enter_context(tc.tile_pool(name="outer", bufs=1))
        dram = es.enter_context(tc.tile_pool(name="dram", bufs=1, space="DRAM"))

        cst = outer.tile([128, 128], F32R, tag="cst", name="cst")      # all 1.0
        cmean = outer.tile([128, 2], F32R, tag="cmean", name="cmean")  # 1/DIM, eps
        cstb = outer.tile([128, 16], BF16, tag="cstb", name="cstb")    # ones bf16
        fmask = outer.tile([128, 1], F32R, tag="fmask", name="fmask")
        nc.sync.dma_start(cst[:], onesin[:])
        nc.sync.dma_start(cmean[:], cmeanin[:])
        nc.sync.dma_start(fmask[:], fmaskin[:])
        nc.scalar.copy(cstb[:], cst[:, 0:16])

        # residual stream, dim-major: xt[dc] = x[dc*128:(dc+1)*128, tok]
        xt = [outer.tile([128, T_LOC], F32R, tag=f"xt{d}", name=f"xt{d}")
              for d in range(8)]
        for dc in range(8):
            nc.sync.dma_start(xt[dc][:], tokr[:, dc, :])

        # ------------------------------------------------------------------
        def layer_norm(dst_tiles, sqp, psln):
            """dst[dc] = LN(xt)[dc] in bf16 (gamma folded into weights)."""
            mean_ps = psln.tile([1, T_LOC], F32, tag="mean", name="mean")
            sq_ps = psln.tile([1, T_LOC], F32, tag="sq", name="sq")
            for dc in range(8):
                sq_sb = sqp.tile([128, T_LOC], F32R, tag="sqt", name="sqt")
                nc.scalar.activation(sq_sb[:], xt[dc][:], AF.Square)
                for (toff, tsz) in TC:
                    nc.tensor.matmul(
                        mean_ps[:, toff:toff + tsz], cmean[:, 0:1],
                        xt[dc][:, toff:toff + tsz],
                        start=(dc == 0), stop=(dc == 7),
                    )
                    nc.tensor.matmul(
                        sq_ps[:, toff:toff + tsz], cmean[:, 0:1],
                        sq_sb[:, toff:toff + tsz],
                        start=(dc == 0), stop=(dc == 7),
                    )
            mean_sb = sqp.tile([1, T_LOC], F32R, tag="meansb", name="meansb")
            nc.scalar.copy(mean_sb[:], mean_ps[:])
            m2 = sqp.tile([1, T_LOC], F32, tag="m2", name="m2")
            nc.scalar.activation(m2[:], mean_ps[:], AF.Square)
            var = sqp.tile([1, T_LOC], F32, tag="var", name="var")
            nc.vector.tensor_tensor(out=var[:], in0=sq_ps[:], in1=m2[:],
                                    op=OP.subtract)
            srt = sqp.tile([1, T_LOC], F32, tag="srt", name="srt")
            nc.scalar.activation(srt[:], var[:], AF.Sqrt, bias=cmean[0:1, 1:2])
            rstd = sqp.tile([1, T_LOC], F32R, tag="rstd", name="rstd")
            nc.vector.reciprocal(rstd[:], srt[:])
            mr = sqp.tile([1, T_LOC], F32R, tag="mr", name="mr")
            nc.vector.tensor_tensor(out=mr[:], in0=mean_sb[:], in1=rstd[:],
                                    op=OP.mult)
            # broadcast the [1,520] rows to [128,520] (ones-stationary matmul)
            rs_b = psln.tile([128, T_LOC], F32, tag="rsb", name="rsb")
            mr_b = psln.tile([128, T_LOC], F32, tag="mrb", name="mrb")
            for (toff, tsz) in TC:
                nc.tensor.matmul(rs_b[:, toff:toff + tsz], cst[0:1, 0:128],
                                 rstd[0:1, toff:toff + tsz], start=True, stop=True)
                nc.tensor.matmul(mr_b[:, toff:toff + tsz], cst[0:1, 0:128],
                                 mr[0:1, toff:toff + tsz], start=True, stop=True)
            for dc in range(8):
                tmp = sqp.tile([128, T_LOC], F32R, tag="lntmp", name="lntmp")
                nc.vector.tensor_tensor(out=tmp[:], in0=xt[dc][:], in1=rs_b[:],
                                        op=OP.mult)
                nc.vector.tensor_tensor(out=dst_tiles[dc][:], in0=tmp[:],
                                        in1=mr_b[:], op=OP.subtract)

        def cp(i, dst, src):
            """Alternate psum->sbuf copies between DVE and ACT."""
            if i % 2 == 0:
                nc.vector.tensor_copy(dst, src)
            else:
                nc.scalar.copy(dst, src)

        # ------------------------------------------------------------------
        for l in range(depth):
            with tc.tile_pool(name="qa", bufs=1) as qa:
                # xloc: LN output for local tokens (query side); xg: gathered
                # LN output for both halves in global order (key/value side)
                xloc = [qa.tile([128, T_LOC], BF16, tag=f"xl{d}", name=f"xl{d}")
                        for d in range(8)]
                xg = [[qa.tile([128, T_LOC], BF16, tag=f"xg{h}_{d}",
                               name=f"xg{h}_{d}") for d in range(8)]
                      for h in range(2)]
                qt = [qa.tile([128, T_LOC], BF16, tag=f"qt{m}", name=f"qt{m}")
                      for m in range(8)]
                ktg = [qa.tile([128, T], BF16, tag=f"ktg{m}", name=f"ktg{m}")
                       for m in range(8)]
                vg = [[qa.tile([128, VCOLS], BF16, tag=f"vg{h}_{c}",
                               name=f"vg{h}_{c}")
                       for c in range(5 if h == 0 else 4)] for h in range(2)]
                vfus = qa.tile([16, VCOLS], BF16, tag="vfus", name="vfus")
                att = [qa.tile([128, T_LOC], BF16, tag=f"att{g}", name=f"att{g}")
                       for g in range(8)]
                scratch = qa.tile([65, T_LOC], F32R, tag="scratch", name="scratch")
                bounce = dram.tile([8 * XGB], BF16, tag="bounce", name="bounce")
                gath = dram.tile([16 * XGB], BF16, tag="gath", name="gath")

                with tc.tile_pool(name="lnp", bufs=2) as sqp, \
                     tc.tile_pool(name="ps_ln", bufs=1, space="PSUM") as psln:
                    layer_norm(xloc, sqp, psln)
                # bounce write on the ACT queue: keeps the SP queue free for
                # weight prefetch
                for dc in range(8):
                    nc.scalar.dma_start(
                        bounce[dc * XGB:(dc + 1) * XGB]
                        .rearrange("(p t) -> p t", t=T_LOC), xloc[dc][:])

                nc.gpsimd.collective_compute(
                    "AllGather", mybir.AluOpType.bypass,
                    replica_groups=rg,
                    ins=[bounce.opt()], outs=[gath.opt()],
                )
                # unpack on the Pool queue (idle behind the collective)
                for h in range(2):
                    for dc in range(8):
                        base = (h * 8 + dc) * XGB
                        nc.gpsimd.dma_start(
                            xg[h][dc][:],
                            gath[base:base + XGB]
                            .rearrange("(p t) -> p t", t=T_LOC))

                with tc.tile_pool(name="wstr", bufs=3) as wstr, \
                     tc.tile_pool(name="ps_qkv", bufs=2, space="PSUM") as psq:
                    # ---- q projection (from local xn; overlaps collective)
                    for mc in range(8):
                        wqt = wstr.tile([128, DIM], BF16, tag="wqt", name="wqt")
                        nc.sync.dma_start(wqt[:], wq[l, mc])
                        qp = psq.tile([128, T_LOC], F32, tag="qk_ps",
                                      name="qk_ps")
                        for dc in range(8):
                            for (toff, tsz) in TC:
                                nc.tensor.matmul(
                                    qp[:, toff:toff + tsz],
                                    wqt[:, dc * 128:(dc + 1) * 128],
                                    xloc[dc][:, toff:toff + tsz],
                                    start=(dc == 0), stop=(dc == 7),
                                )
                        cp(mc, qt[mc][:], qp[:])

                    # ---- k projection, all 1040 keys (global order)
                    for mc in range(8):
                        wkt = wstr.tile([128, DIM], BF16, tag="wkt", name="wkt")
                        nc.sync.dma_start(wkt[:], wk[l, mc])
                        for h in range(2):
                            kp = psq.tile([128, T_LOC], F32, tag="qk_ps",
                                          name="qk_ps")
                            for dc in range(8):
                                for (toff, tsz) in TC:
                                    nc.tensor.matmul(
                                        kp[:, toff:toff + tsz],
                                        wkt[:, dc * 128:(dc + 1) * 128],
                                        xg[h][dc][:, toff:toff + tsz],
                                        start=(dc == 0), stop=(dc == 7),
                                    )
                            cp(mc + h, ktg[mc][:, h * T_LOC:(h + 1) * T_LOC],
                               kp[:])

                # ---- v projection, all keys; wvd[dc] live across chunks
                with tc.tile_pool(name="wvp", bufs=1) as wvp, \
                     tc.tile_pool(name="ps_vb", bufs=2, space="PSUM") as psvb, \
                     tc.tile_pool(name="ps_vt", bufs=1, space="PSUM") as psvt:
                    wvd = [wvp.tile([128, VCOLS], BF16, tag=f"wvd{d}",
                                    name=f"wvd{d}") for d in range(8)]
                    for dc in range(8):
                        nc.sync.dma_start(wvd[dc][:], wv[l, dc])
                    vjobs = [(0, ci, koff, ksz)
                             for ci, (koff, ksz) in enumerate(TCH)]
                    vjobs += [(1, ci, koff, ksz)
                              for ci, (koff, ksz) in enumerate(TCH1)]
                    vjobs.append((1, -1, 504, 16))    # fusion keys
                    for (h, ci, koff, ksz) in vjobs:
                        dstt = vfus if ci < 0 else vg[h][ci]
                        vp = psvb.tile([128, 1024], F32, tag="v_ps", name="v_ps")
                        vp8 = psvt.tile([128, 16], F32, tag="v_ps8", name="v_ps8")
                        for dc in range(8):
                            for vo in (0, 512):
                                nc.tensor.matmul(
                                    vp[0:ksz, vo:vo + 512],
                                    xg[h][dc][:, koff:koff + ksz],
                                    wvd[dc][:, vo:vo + 512],
                                    start=(dc == 0), stop=(dc == 7),
                                )
                            nc.tensor.matmul(
                                vp8[0:ksz, :],
                                xg[h][dc][:, koff:koff + ksz],
                                wvd[dc][:, 1024:1040],
                                start=(dc == 0), stop=(dc == 7),
                            )
                        cp(ci, dstt[0:ksz, 0:1024], vp[0:ksz, :])
                        cp(ci + 1, dstt[0:ksz, 1024:1040], vp8[0:ksz, :])
                        # ones column per head (softmax denominator)
                        v3 = dstt[0:ksz, :].rearrange(
                            "p (g c) -> p g c", c=DH + 1)
                        nc.scalar.copy(v3[:, :, DH:DH + 1],
                                       cstb[0:ksz, 0:NH].unsqueeze(-1))

                # ---- attention ------------------------------------------
                with tc.tile_pool(name="attw", bufs=2) as attw, \
                     ExitStack() as es_att:
                    # fusion-key pre-pass: keys 1024..1040 for local queries
                    # 504..520, scaled later by fmask (1 only on odd cores)
                    fus_sb = []
                    with tc.tile_pool(name="ps_fus", bufs=2, space="PSUM") as psF:
                        for hh in range(NH):
                            g, pb = hh // 2, (hh % 2) * 64
                            sf = psF.tile([16, 16], F32, tag="fus_s", name="fus_s")
                            nc.tensor.matmul(
                                sf[:],
                                ktg[g][pb:pb + 64, T_LOC + 504:T_LOC + 520],
                                qt[g][pb:pb + 64, 504:520], start=True, stop=True)
                            ef = attw.tile([16, 16], BF16, tag="fus_e",
                                           name="fus_e")
                            nc.scalar.activation(ef[:], sf[:], AF.Exp, scale=0.125)
                            of = psF.tile([65, 16], F32, tag="fus_o", name="fus_o")
                            nc.tensor.matmul(
                                of[:], vfus[:, hh * 65:(hh + 1) * 65], ef[:],
                                start=True, stop=True)
                            ft = qa.tile([65, 16], F32, tag=f"fus{hh}",
                                         name=f"fus{hh}")
                            nc.scalar.copy(ft[:], of[:])
                            fus_sb.append(ft)

                    psS = es_att.enter_context(
                        tc.tile_pool(name="ps_s", bufs=2, space="PSUM"))
                    psO = es_att.enter_context(
                        tc.tile_pool(name="ps_o", bufs=1, space="PSUM"))
                    psD = es_att.enter_context(
                        tc.tile_pool(name="ps_d", bufs=1, space="PSUM"))
                    for hh in range(NH):
                        g, pb = hh // 2, (hh % 2) * 64
                        ou = psO.tile([65, T_LOC], F32, tag="ou", name="ou")
                        for ji, (r, joff, jsz) in enumerate(JCH):
                            first, last = (ji == 0), (ji == len(JCH) - 1)
                            goff = r * T_LOC + joff
                            ex = attw.tile([128, T_LOC], BF16, tag="exp",
                                           name="exp")
                            s_a = psS.tile([128, T_LOC], F32, tag="simA",
                                           name="simA")
                            vtile = vg[r][joff // 128]
                            for (toff, tsz) in TC:
                                nc.tensor.matmul(
                                    s_a[0:jsz, toff:toff + tsz],
                                    ktg[g][pb:pb + 64, goff:goff + jsz],
                                    qt[g][pb:pb + 64, toff:toff + tsz],
                                    start=True, stop=True)
                            nc.scalar.activation(ex[0:jsz, :], s_a[0:jsz, :],
                                                 AF.Exp, scale=0.125)
                            for (toff, tsz) in TC:
                                nc.tensor.matmul(
                                    ou[:, toff:toff + tsz],
                                    vtile[0:jsz, hh * 65:(hh + 1) * 65],
                                    ex[0:jsz, toff:toff + tsz],
                                    start=first, stop=last)
                        ou_sb = attw.tile([65, T_LOC], F32, tag="ousb",
                                          name="ousb")
                        nc.vector.tensor_copy(ou_sb[:], ou[:])
                        nc.vector.scalar_tensor_tensor(
                            out=ou_sb[:, 504:520], in0=fus_sb[hh][:],
                            scalar=fmask[0:65, 0:1], in1=ou_sb[:, 504:520],
                            op0=OP.mult, op1=OP.add)
                        nc.vector.reciprocal(scratch[64:65, :],
                                             ou_sb[64:65, :])
                        db512 = psD.tile([64, 512], F32, tag="db512",
                                         name="db512")
                        db8 = psD.tile([64, 8], F32, tag="db8", name="db8")
                        nc.tensor.matmul(db512[:], cst[64:65, 0:64],
                                         scratch[64:65, 0:512],
                                         start=True, stop=True)
                        nc.tensor.matmul(db8[:], cst[64:65, 0:64],
                                         scratch[64:65, 512:520],
                                         start=True, stop=True)
                        db_sb = attw.tile([64, T_LOC], F32, tag="dbsb",
                                          name="dbsb")
                        nc.scalar.copy(db_sb[:, 0:512], db512[:])
                        nc.scalar.copy(db_sb[:, 512:520], db8[:])
                        if hh % 2 == 0:
                            nc.vector.tensor_tensor(
                                out=att[g][0:64, :], in0=ou_sb[0:64, :],
                                in1=db_sb[:], op=OP.mult)
                        else:
                            tmp = attw.tile([64, T_LOC], BF16, tag="atmp",
                                            name="atmp")
                            nc.vector.tensor_tensor(
                                out=tmp[:], in0=ou_sb[0:64, :], in1=db_sb[:],
                                op=OP.mult)
                            nc.scalar.copy(att[g][64:128, :], tmp[:])

                # ---- Wo projection + residual ---------------------------
                with tc.tile_pool(name="wop", bufs=3) as wop, \
                     tc.tile_pool(name="ps_wo", bufs=2, space="PSUM") as psW:
                    for fc in range(8):
                        wot = wop.tile([128, DIM], BF16, tag="wot", name="wot")
                        nc.sync.dma_start(wot[:], wo[l, fc])
                        pp = psW.tile([128, T_LOC], F32, tag="proj", name="proj")
                        for ic in range(8):
                            for (toff, tsz) in TC:
                                nc.tensor.matmul(
                                    pp[:, toff:toff + tsz],
                                    wot[:, ic * 128:(ic + 1) * 128],
                                    att[ic][:, toff:toff + tsz],
                                    start=(ic == 0), stop=(ic == 7))
                        nc.vector.tensor_tensor(
                            out=xt[fc][:], in0=xt[fc][:], in1=pp[:], op=OP.add)

            # ---------------- FFN ----------------
            with tc.tile_pool(name="ff", bufs=1) as ffp, \
                 tc.tile_pool(name="ffw", bufs=3) as ffw, \
                 tc.tile_pool(name="sq2", bufs=2) as sqp2:
                xn2 = [ffp.tile([128, T_LOC], BF16, tag=f"xn2_{d}",
                                name=f"xn2_{d}") for d in range(8)]
                with tc.tile_pool(name="ps_ln2", bufs=1, space="PSUM") as psln2:
                    layer_norm(xn2, sqp2, psln2)

                ffa = [ffp.tile([128, T_LOC], BF16, tag=f"ffa{m}", name=f"ffa{m}")
                       for m in range(FFC)]
                with tc.tile_pool(name="ps_ff1", bufs=2, space="PSUM") as psg2:
                    for mc in range(FFC):
                        w1xt = ffw.tile([128, DIM], BF16, tag="w1xt", name="w1xt")
                        w1gt = ffw.tile([128, DIM], BF16, tag="w1gt", name="w1gt")
                        nc.sync.dma_start(w1xt[:], w1x[l, mc])
                        nc.sync.dma_start(w1gt[:], w1g[l, mc])
                        xh_ps = psg2.tile([128, T_LOC], F32, tag="xh", name="xh")
                        gt_ps = psg2.tile([128, T_LOC], F32, tag="gt", name="gt")
                        for dc in range(8):
                            for (toff, tsz) in TC:
                                nc.tensor.matmul(
                                    xh_ps[:, toff:toff + tsz],
                                    w1xt[:, dc * 128:(dc + 1) * 128],
                                    xn2[dc][:, toff:toff + tsz],
                                    start=(dc == 0), stop=(dc == 7))
                                nc.tensor.matmul(
                                    gt_ps[:, toff:toff + tsz],
                                    w1gt[:, dc * 128:(dc + 1) * 128],
                                    xn2[dc][:, toff:toff + tsz],
                                    start=(dc == 0), stop=(dc == 7))
                        gel = sqp2.tile([128, T_LOC], F32R, tag="gel", name="gel")
                        nc.scalar.activation(gel[:], gt_ps[:], AF.Gelu)
                        nc.vector.tensor_tensor(
                            out=ffa[mc][:], in0=xh_ps[:], in1=gel[:],
                            op=OP.mult)

                w2r = [ffp.tile([128, DIM], BF16, tag=f"w2r{i}", name=f"w2r{i}")
                       for i in range(FFC)]
                for ic in range(FFC):
                    nc.sync.dma_start(w2r[ic][:], w2[l, ic])
                with tc.tile_pool(name="ps_ff2", bufs=1, space="PSUM") as pst2:
                    for fblk in range(2):
                        fps = [pst2.tile([128, T_LOC], F32, tag=f"f2_{i}",
                                         name=f"f2_{i}") for i in range(4)]
                        for ic in range(FFC):
                            for i in range(4):
                                fc = fblk * 4 + i
                                for (toff, tsz) in TC:
                                    nc.tensor.matmul(
                                        fps[i][:, toff:toff + tsz],
                                        w2r[ic][:, fc * 128:(fc + 1) * 128],
                                        ffa[ic][:, toff:toff + tsz],
                                        start=(ic == 0), stop=(ic == FFC - 1))
                        for i in range(4):
                            fc = fblk * 4 + i
                            nc.vector.tensor_tensor(
                                out=xt[fc][:], in0=xt[fc][:], in1=fps[i][:],
                                op=OP.add)

        # ------------------------------------------------------------------
        # final LN (no gamma; applied on host) + token-sum
        with tc.tile_pool(name="fin", bufs=1) as finp, \
             tc.tile_pool(name="sqf", bufs=2) as sqpf, \
             tc.tile_pool(name="ps_lnf", bufs=1, space="PSUM") as pslnf:
            xnf = [finp.tile([128, T_LOC], BF16, tag=f"xnf{d}", name=f"xnf{d}")
                   for d in range(8)]
            layer_norm(xnf, sqpf, pslnf)
            for dc in range(8):
                s = sqpf.tile([128, 1], F32, tag="osum", name="osum")
                nc.vector.reduce_sum(s[:], xnf[dc][:], axis=AX.X)
                nc.sync.dma_start(
                    osum[dc * 128:(dc + 1) * 128].rearrange("(p o) -> p o", o=1),
                    s[:])

    _split_multi_waits(nc)
    return nc


def prepare_inputs(inputs, depth=DEPTH):
    """Build the 8 per-core input maps from the full problem inputs."""
    import ml_dtypes
    bf16 = ml_dtypes.bfloat16
    f32 = np.float32
    emb = np.asarray(inputs["emb"], f32)
    fus = np.asarray(inputs["fusion_tokens"], f32)
    embt = np.concatenate([emb, fus], axis=0)
    idx_full = np.asarray(inputs["spliced_index"], np.int32)
    dat_full = np.asarray(inputs["spliced_data"], f32)

    g1 = np.asarray(inputs["ln1_g"], f32)[:depth]          # [depth, DIM]
    g2 = np.asarray(inputs["ln2_g"], f32)[:depth]

    wq_full = np.asarray(inputs["Wq"], f32)[:depth] * g1[:, :, None]
    wkv = np.asarray(inputs["Wkv"], f32)[:depth] * g1[:, :, None]
    wo_full = np.asarray(inputs["Wo"], f32)[:depth]
    w1_full = np.asarray(inputs["Wff1"], f32)[:depth] * g2[:, :, None]
    w2_full = np.asarray(inputs["Wff2"], f32)[:depth]
    wk_full = wkv[:, :, :DIM]
    wv_raw = wkv[:, :, DIM:]
    # v columns interleaved: per head 64 cols + 1 pad col (ones set on-chip)
    wv_full = np.zeros((depth, DIM, VCOLS), f32)
    wvh = wv_raw.reshape(depth, DIM, NH, DH)
    wv_full.reshape(depth, DIM, NH, DH + 1)[:, :, :, :DH] = wvh

    def tile_kmajor(w, n_in_chunks, cols):
        # w: [depth, K, cols] -> [depth, n_in_chunks, 128, cols]
        d = w.shape[0]
        return np.ascontiguousarray(
            w.reshape(d, n_in_chunks, 128, cols)).astype(bf16)

    def tile_outmajor(w, n_out_chunks):
        # w: [depth, K, n_out*128] -> [depth, mc, 128, K] with
        # [l, mc, p, kc*128 + c] = w[l, kc*128 + p, mc*128 + c]
        d, K, _ = w.shape
        nk = K // 128
        w5 = w.reshape(d, nk, 128, n_out_chunks, 128)
        w5 = w5.transpose(0, 3, 2, 1, 4)
        return np.ascontiguousarray(
            w5.reshape(d, n_out_chunks, 128, nk * 128)).astype(bf16)

    w1xp = np.zeros((depth, DIM, FF_PAD), f32)
    w1gp = np.zeros((depth, DIM, FF_PAD), f32)
    w1xp[:, :, :FF] = w1_full[:, :, :FF]
    w1gp[:, :, :FF] = w1_full[:, :, FF:]
    w2p = np.zeros((depth, FF_PAD, DIM), f32)
    w2p[:, :FF, :] = w2_full

    ones = np.ones((128, 128), f32)
    cmean = np.zeros((128, 2), f32)
    cmean[:, 0] = 1.0 / DIM
    cmean[:, 1] = 1e-5

    shared = dict(
        onesin=ones, cmeanin=cmean,
        wq=tile_outmajor(wq_full, 8),
        wk=tile_outmajor(wk_full, 8),
        wv=tile_kmajor(wv_full, 8, VCOLS),
        wo=tile_outmajor(wo_full, 8),
        w1x=tile_outmajor(w1xp, FFC),
        w1g=tile_outmajor(w1gp, FFC),
        w2=tile_kmajor(w2p, FFC, DIM),
    )

    in_maps = []
    for c in range(8):
        b, r = c // 2, c % 2
        if r == 0:
            idx = idx_full[b, 0:T_LOC].astype(np.int64)
            w = dat_full[b, 0:T_LOC]
        else:
            spl = idx_full[b, T_LOC:N_SPL].astype(np.int64)   # 504 tokens
            idx = np.concatenate(
                [spl, np.arange(VOCAB, VOCAB + NF, dtype=np.int64)])
            w = np.concatenate([dat_full[b, T_LOC:N_SPL], np.ones(NF, f32)])
        tok = (embt[idx] * w[:, None]).astype(f32)        # [520, DIM]
        tokr = np.ascontiguousarray(
            tok.T.reshape(8, 128, T_LOC).transpose(1, 0, 2))  # [128, 8, 520]
        fmask = np.full((128, 1), float(r), f32)
        in_maps.append(dict(shared, tokr=tokr, fmaskin=fmask))
    return in_maps


def epilogue(osums, inputs):
    """Host epilogue: final-LN gamma, pooled attention (uniform mean over v)
    + contrastive loss."""
    f64 = np.float64
    pWkv = np.asarray(inputs["pWkv"], f64)
    pWo = np.asarray(inputs["pWo"], f64)
    ret = np.asarray(inputs["return_tokens"], f64)
    norm_g = np.asarray(inputs["norm_g"], f64)
    lsc = float(np.asarray(inputs["logit_scale_c"]))
    lsf = float(np.asarray(inputs["logit_scale_f"]))

    meantok = np.stack([
        (osums[2 * b].astype(f64) + osums[2 * b + 1].astype(f64)) * norm_g / T
        for b in range(B)
    ])  # [B, DIM]
    mv = meantok @ pWkv[:, NH * DH:]
    pooled_pre = mv @ pWo
    spliced = pooled_pre + ret[0]
    fusion = pooled_pre + ret[1]

    def closs(a, bv, ls):
        an = a / np.linalg.norm(a, axis=-1, keepdims=True)
        bn = bv / np.linalg.norm(bv, axis=-1, keepdims=True)
        lg = np.exp(ls) * (an @ bn.T)

        def nll(m):
            mx = m.max(-1, keepdims=True)
            lse = mx + np.log(np.exp(m - mx).sum(-1, keepdims=True))
            return -np.diag(m - lse).mean()

        return (nll(lg) + nll(lg.T)) * 0.5

    loss = closs(spliced, spliced, lsc) + closs(spliced, fusion, lsf)
    return np.float32(loss)


def _get_program():
    if "nc" not in _CACHE:
        _CACHE["nc"] = build_program(DEPTH)
    return _CACHE["nc"]


def kernel(**inputs):
    from concourse.bass_utils import run_bass_kernel_spmd

    nc = _get_program()
    in_maps = prepare_inputs(inputs, DEPTH)
    res = run_bass_kernel_spmd(nc, in_maps, list(range(8)))
    osums = [res.results[c]["osum"] for c in range(8)]
    return epilogue(osums, inputs)
